# revision 30
# baseline (speedup 1.0000x reference)
"""Binary-tree gated-expert MoE (root -> 2 mid -> 4 leaf experts) on 8 trn2 cores.

Strategy: expert-parallel dispatch by leaf index. Tokens are grouped on the
host by their 2-bit routing path (leaf = 2*bit0 + bit1); each of the 8
NeuronCores processes one contiguous chunk of one leaf's tokens (cores are
apportioned to leaves proportionally to token counts, 2 cores/leaf in the
balanced case). A core then runs 3 chained dense [C,2048]x[2048,2048] layers
(root W0, mid W1[bit0], leaf W2[leaf]) with relu+bias, entirely on-chip.

Device kernel keeps activations transposed ([D, tokens] feature-major) so each
layer's matmul output (PSUM [fout, tok]) is directly the next layer's rhs.
Matmuls run in fp16 (same TensorE rate as bf16, 8x finer mantissa) with fp32
PSUM accumulation; weights are streamed from HBM as pre-tiled [16, 128, 2048]
stripes and used as the stationary operand.
"""

import numpy as np
from contextlib import ExitStack

import concourse.bass as bass
from concourse import bacc, mybir, tile
from concourse.bass_utils import run_bass_kernel_spmd

D = 2048
PT = 128           # partition tile
KT = D // PT       # 16 contraction tiles per layer
MT = D // PT       # 16 output-feature tiles per layer
N_CORES = 8

F32 = mybir.dt.float32
F16 = mybir.dt.float16
NP_F16 = np.float16

# cache of compiled bass programs keyed by padded capacity C
_compiled = {}
# stash of the last run's results so a harness can inspect exec_time_ns
last_results = None


def _prep_weight(W):
    """[D, D] -> [MT, 128, D] bf16: stripe m holds W[:, m*128:(m+1)*128]
    rearranged so partition p = contraction row within k-chunk, and the free
    dim is (k, fout-col) — i.e. out[m, p, k*128 + c] = W[k*128 + p, m*128 + c].
    Each [128, 2048] stripe then DMAs contiguously into SBUF and its k-th
    [128, 128] column block is exactly the lhsT (stationary) matmul operand."""
    W4 = W.reshape(KT, PT, MT, PT)
    return np.ascontiguousarray(
        W4.transpose(2, 1, 0, 3).reshape(MT, PT, D).astype(NP_F16)
    )


def _prep_bias(b0, b1e, b2l):
    """three [D] biases -> [128, 3*MT] f32 where column li*MT + m holds
    bias[li][m*128 : (m+1)*128] along partitions."""
    cols = []
    for b in (b0, b1e, b2l):
        cols.append(b.reshape(MT, PT).T)  # [128, MT]
    return np.ascontiguousarray(np.concatenate(cols, axis=1).astype(np.float32))


def _tiling(maxg):
    """Pick (TN, NT, C): NT token tiles of TN columns, C = NT*TN >= maxg,
    TN <= 512 (one PSUM bank of fp32), minimizing padded capacity C."""
    maxg = max(maxg, 128)
    NT = -(-maxg // 512)
    TN = -(-(-(-maxg // NT)) // 4) * 4
    return TN, NT, TN * NT


def _build(C, TN, NT):
    """Build + compile the 3-layer SPMD program for per-core capacity C.

    Layer-1 matmuls must consume the 16 k-chunks of the input as they stream
    in, so the m loop runs in pairs (6 PSUM tiles live per pair, 8 banks
    total): each pair's k-loop trickles behind the input DMA instead of one
    m-tile waiting for the entire input. Weight stripes ride the scalar
    (qActDynamicHW) DMA ring so they never queue behind the big input
    transfers on the sync (qSPDynamicHW) ring."""
    nc = bacc.Bacc(
        "TRN2",
        target_bir_lowering=False,
        debug=False,
        enable_asserts=False,
        num_devices=N_CORES,
    )
    xT = nc.dram_tensor("xT", [D, C], F16, kind="ExternalInput").ap()
    w0 = nc.dram_tensor("w0", [MT, PT, D], F16, kind="ExternalInput").ap()
    w1 = nc.dram_tensor("w1", [MT, PT, D], F16, kind="ExternalInput").ap()
    w2 = nc.dram_tensor("w2", [MT, PT, D], F16, kind="ExternalInput").ap()
    bias = nc.dram_tensor("bias", [PT, 3 * MT], F32, kind="ExternalInput").ap()
    yT = nc.dram_tensor("yT", [D, C], F32, kind="ExternalOutput").ap()

    with tile.TileContext(nc) as tc, ExitStack() as ctx:
        wpool = ctx.enter_context(tc.tile_pool(name="w", bufs=4))
        hpool = ctx.enter_context(tc.tile_pool(name="h", bufs=1))
        pspool = ctx.enter_context(tc.tile_pool(name="ps", bufs=8, space="PSUM"))
        opool = ctx.enter_context(tc.tile_pool(name="o", bufs=4))
        cpool = ctx.enter_context(tc.tile_pool(name="c", bufs=1))

        hA = hpool.tile([PT, KT, C], F16, tag="hA")
        hB = hpool.tile([PT, KT, C], F16, tag="hB")

        # All early DMAs round-robin across the shared SDMA engines at packet
        # granularity, so emission order ~= bandwidth share. The first matmul
        # needs stripe (w0, m=0) + x chunk 0; stripe m=1 is needed a few
        # hundred ns later; bias only at the first epilogue (~20us in).
        # Split the k=0 slices of stripes m=0,1 and the n=0 columns of x
        # chunk 0 into their own small DMAs: the first matmuls then gate on
        # ~120KB of receipts instead of ~800KB.
        wts0 = []
        for m in (0, 1):
            wt = wpool.tile([PT, D], F16, tag="wt", name=f"wt0_{m}")
            nc.scalar.dma_start(wt[:, 0:PT], w0[m, :, 0:PT])
            wts0.append(wt)
        nc.sync.dma_start(hA[:, 0, 0:TN], xT[0:PT, 0:TN])
        for m in (0, 1):
            nc.scalar.dma_start(wts0[m][:, PT:D], w0[m, :, PT:D])
        nc.sync.dma_start(hA[:, 0, TN:C], xT[0:PT, TN:C])
        # chunks 1-3 land inside the pair-0 stall window; split them so the
        # next k-sweep gates on a partial receipt
        for k in range(1, 4):
            nc.sync.dma_start(
                hA[:, k, 0 : 2 * TN], xT[k * PT : (k + 1) * PT, 0 : 2 * TN]
            )
            nc.sync.dma_start(
                hA[:, k, 2 * TN : C], xT[k * PT : (k + 1) * PT, 2 * TN : C]
            )
        for k in range(4, KT):
            nc.sync.dma_start(hA[:, k, :], xT[k * PT : (k + 1) * PT, :])
        bias_sb = cpool.tile([PT, 3 * MT], F32)
        nc.scalar.dma_start(bias_sb[:], bias[:])

        def relu_bias(out_ap, ps_ap, b_ap, on_dve):
            if on_dve:
                nc.vector.tensor_scalar(
                    out_ap, ps_ap, b_ap, 0.0,
                    mybir.AluOpType.add, mybir.AluOpType.max,
                )
            else:
                nc.scalar.activation(
                    out_ap, ps_ap,
                    mybir.ActivationFunctionType.Relu, bias=b_ap,
                )

        layers = [(w0, 0, hA, hB), (w1, 1, hB, hA), (w2, 2, hA, None)]
        for w_dram, li, h_in, h_out in layers:
            for mp in range(MT // 2):
                ms = (2 * mp, 2 * mp + 1)
                if li == 0 and mp == 0:
                    wts = wts0
                else:
                    wts = []
                    for m in ms:
                        wt = wpool.tile([PT, D], F16, tag="wt", name=f"wt{li}_{m}")
                        nc.scalar.dma_start(wt[:], w_dram[m])
                        wts.append(wt)
                pss = {
                    (m, n): pspool.tile([PT, TN], F32, tag="ps", name=f"ps{li}_{m}_{n}")
                    for m in ms
                    for n in range(NT)
                }

                def epilogue(mi, m, n):
                    b_ap = bias_sb[:, li * MT + m : li * MT + m + 1]
                    # alternate ACT/DVE so epilogues drain on two engines
                    on_dve = (n + mi) % 2 == 1
                    if h_out is not None:
                        relu_bias(
                            h_out[:, m, bass.ts(n, TN)], pss[(m, n)][:],
                            b_ap, on_dve,
                        )
                    else:
                        ot = opool.tile([PT, TN], F32, tag="ot", name=f"ot{m}_{n}")
                        relu_bias(ot[:], pss[(m, n)][:], b_ap, on_dve)
                        dma_eng = nc.sync if on_dve else nc.scalar
                        dma_eng.dma_start(
                            yT[m * PT : (m + 1) * PT, bass.ts(n, TN)], ot[:]
                        )

                if li == 0 and mp == 0:
                    # k-outer: consume the streaming input chunks as they land
                    for k in range(KT):
                        for mi, m in enumerate(ms):
                            for n in range(NT):
                                nc.tensor.matmul(
                                    pss[(m, n)][:],
                                    wts[mi][:, k * PT : (k + 1) * PT],
                                    h_in[:, k, bass.ts(n, TN)],
                                    start=(k == 0),
                                    stop=(k == KT - 1),
                                    skip_group_check=True,
                                )
                    for mi, m in enumerate(ms):
                        for n in range(NT):
                            epilogue(mi, m, n)
                else:
                    # inputs resident: k-inner per tile, so each tile's
                    # epilogue (and final-layer out-DMA) fires as soon as its
                    # accumulation completes — the kernel tail drains one
                    # tile, not six
                    for mi, m in enumerate(ms):
                        for n in range(NT):
                            for k in range(KT):
                                nc.tensor.matmul(
                                    pss[(m, n)][:],
                                    wts[mi][:, k * PT : (k + 1) * PT],
                                    h_in[:, k, bass.ts(n, TN)],
                                    start=(k == 0),
                                    stop=(k == KT - 1),
                                )
                            if li == 2 and m == MT - 1 and n == NT - 1:
                                # very last tile: drain both halves in
                                # parallel on both engines and both DMA rings
                                b_ap = bias_sb[:, li * MT + m : li * MT + m + 1]
                                h2 = TN // 2
                                ot = opool.tile([PT, TN], F32, tag="ot", name="ot_last")
                                ps_ap = pss[(m, n)]
                                rows = slice(m * PT, (m + 1) * PT)
                                nc.scalar.activation(
                                    ot[:, 0:h2], ps_ap[:, 0:h2],
                                    mybir.ActivationFunctionType.Relu, bias=b_ap,
                                )
                                nc.vector.tensor_scalar(
                                    ot[:, h2:TN], ps_ap[:, h2:TN], b_ap, 0.0,
                                    mybir.AluOpType.add, mybir.AluOpType.max,
                                )
                                nc.sync.dma_start(
                                    yT[rows, n * TN : n * TN + h2], ot[:, 0:h2]
                                )
                                nc.scalar.dma_start(
                                    yT[rows, n * TN + h2 : (n + 1) * TN], ot[:, h2:TN]
                                )
                            else:
                                epilogue(mi, m, n)
    nc.compile()
    return nc


def _apportion_cores(counts):
    """Assign 8 cores to 4 leaves ~proportionally to token counts.
    Returns list of core counts per leaf (sums to N_CORES; 0 only for empty
    leaves). Greedy: repeatedly hand a core to the leaf with max load/core."""
    alive = [l for l in range(4) if counts[l] > 0]
    n = {l: 1 for l in alive}
    for _ in range(N_CORES - len(alive)):
        l = max(alive, key=lambda l: counts[l] / n[l])
        n[l] += 1
    return [n.get(l, 0) for l in range(4)]


def kernel(x, W0, b0, W1, b1, W2, b2, path_mask):
    global last_results
    x = np.asarray(x, dtype=np.float32)
    path_mask = np.asarray(path_mask)
    W0, b0, W1, b1, W2, b2 = (
        np.asarray(a, dtype=np.float32) for a in (W0, b0, W1, b1, W2, b2)
    )
    B = x.shape[0]

    bit0 = path_mask[:, 0].astype(np.int64)
    bit1 = path_mask[:, 1].astype(np.int64)
    leaf = 2 * bit0 + bit1
    order = np.argsort(leaf, kind="stable")
    counts = np.bincount(leaf, minlength=4)

    per_leaf = _apportion_cores(counts)
    # contiguous chunks of the leaf-sorted order per core
    groups = []      # list of (leaf, index-array) per core
    start = 0
    for l in range(4):
        cnt = int(counts[l])
        tok = order[start : start + cnt]
        start += cnt
        nl = per_leaf[l]
        if nl == 0:
            continue
        bounds = [round(i * cnt / nl) for i in range(nl + 1)]
        for i in range(nl):
            groups.append((l, tok[bounds[i] : bounds[i + 1]]))
    while len(groups) < N_CORES:  # only if some leaf was empty and slots remain
        groups.append((0, np.zeros(0, dtype=np.int64)))

    maxg = max(len(g[1]) for g in groups)
    TN, NT, C = _tiling(maxg)

    if C not in _compiled:
        _compiled[C] = _build(C, TN, NT)
    nc = _compiled[C]

    w_prepped = {}  # cache per (matrix id)
    def wp(tag, W):
        if tag not in w_prepped:
            w_prepped[tag] = _prep_weight(W)
        return w_prepped[tag]

    xb = x.astype(NP_F16)
    in_maps = []
    for l, tok in groups:
        xTg = np.zeros((D, C), dtype=NP_F16)
        if len(tok):
            xTg[:, : len(tok)] = xb[tok].T
        in_maps.append(
            {
                "xT": xTg,
                "w0": wp("w0", W0),
                "w1": wp(("w1", l // 2), W1[l // 2]),
                "w2": wp(("w2", l), W2[l]),
                "bias": _prep_bias(b0, b1[l // 2], b2[l]),
            }
        )

    last_results = run_bass_kernel_spmd(nc, in_maps, core_ids=list(range(N_CORES)))

    y = np.empty((B, D), dtype=np.float32)
    for (l, tok), res in zip(groups, last_results.results):
        if len(tok):
            y[tok] = res["yT"][:, : len(tok)].T
    return y


# revision 32
# speedup vs baseline: 1.0053x; 1.0053x over previous
"""Binary-tree gated-expert MoE (root -> 2 mid -> 4 leaf experts) on 8 trn2 cores.

Strategy: expert-parallel dispatch by leaf index. Tokens are grouped on the
host by their 2-bit routing path (leaf = 2*bit0 + bit1); each of the 8
NeuronCores processes one contiguous chunk of one leaf's tokens (cores are
apportioned to leaves proportionally to token counts, 2 cores/leaf in the
balanced case). A core then runs 3 chained dense [C,2048]x[2048,2048] layers
(root W0, mid W1[bit0], leaf W2[leaf]) with relu+bias, entirely on-chip.

Device kernel keeps activations transposed ([D, tokens] feature-major) so each
layer's matmul output (PSUM [fout, tok]) is directly the next layer's rhs.
Matmuls run in fp16 (same TensorE rate as bf16, 8x finer mantissa) with fp32
PSUM accumulation; weights are streamed from HBM as pre-tiled [16, 128, 2048]
stripes and used as the stationary operand.
"""

import numpy as np
from contextlib import ExitStack

import concourse.bass as bass
from concourse import bacc, mybir, tile
from concourse.bass_utils import run_bass_kernel_spmd

D = 2048
PT = 128           # partition tile
KT = D // PT       # 16 contraction tiles per layer
MT = D // PT       # 16 output-feature tiles per layer
N_CORES = 8

F32 = mybir.dt.float32
F16 = mybir.dt.float16
NP_F16 = np.float16

# cache of compiled bass programs keyed by padded capacity C
_compiled = {}
# stash of the last run's results so a harness can inspect exec_time_ns
last_results = None


def _prep_weight(W):
    """[D, D] -> [MT, 128, D] bf16: stripe m holds W[:, m*128:(m+1)*128]
    rearranged so partition p = contraction row within k-chunk, and the free
    dim is (k, fout-col) — i.e. out[m, p, k*128 + c] = W[k*128 + p, m*128 + c].
    Each [128, 2048] stripe then DMAs contiguously into SBUF and its k-th
    [128, 128] column block is exactly the lhsT (stationary) matmul operand."""
    W4 = W.reshape(KT, PT, MT, PT)
    return np.ascontiguousarray(
        W4.transpose(2, 1, 0, 3).reshape(MT, PT, D).astype(NP_F16)
    )


def _prep_bias(b0, b1e, b2l):
    """three [D] biases -> [128, 3*MT] f32 where column li*MT + m holds
    bias[li][m*128 : (m+1)*128] along partitions."""
    cols = []
    for b in (b0, b1e, b2l):
        cols.append(b.reshape(MT, PT).T)  # [128, MT]
    return np.ascontiguousarray(np.concatenate(cols, axis=1).astype(np.float32))


def _tiling(maxg):
    """Pick (TN, NT, C): NT token tiles of TN columns, C = NT*TN >= maxg,
    TN <= 512 (one PSUM bank of fp32), minimizing padded capacity C."""
    maxg = max(maxg, 128)
    NT = -(-maxg // 512)
    TN = -(-(-(-maxg // NT)) // 4) * 4
    return TN, NT, TN * NT


def _build(C, TN, NT):
    """Build + compile the 3-layer SPMD program for per-core capacity C.

    Layer-1 matmuls must consume the 16 k-chunks of the input as they stream
    in, so the m loop runs in pairs (6 PSUM tiles live per pair, 8 banks
    total): each pair's k-loop trickles behind the input DMA instead of one
    m-tile waiting for the entire input. Weight stripes ride the scalar
    (qActDynamicHW) DMA ring so they never queue behind the big input
    transfers on the sync (qSPDynamicHW) ring."""
    nc = bacc.Bacc(
        "TRN2",
        target_bir_lowering=False,
        debug=False,
        enable_asserts=False,
        num_devices=N_CORES,
    )
    xT = nc.dram_tensor("xT", [D, C], F16, kind="ExternalInput").ap()
    w0 = nc.dram_tensor("w0", [MT, PT, D], F16, kind="ExternalInput").ap()
    w1 = nc.dram_tensor("w1", [MT, PT, D], F16, kind="ExternalInput").ap()
    w2 = nc.dram_tensor("w2", [MT, PT, D], F16, kind="ExternalInput").ap()
    bias = nc.dram_tensor("bias", [PT, 3 * MT], F32, kind="ExternalInput").ap()
    yT = nc.dram_tensor("yT", [D, C], F32, kind="ExternalOutput").ap()

    with tile.TileContext(nc) as tc, ExitStack() as ctx:
        wpool = ctx.enter_context(tc.tile_pool(name="w", bufs=4))
        hpool = ctx.enter_context(tc.tile_pool(name="h", bufs=1))
        pspool = ctx.enter_context(tc.tile_pool(name="ps", bufs=8, space="PSUM"))
        opool = ctx.enter_context(tc.tile_pool(name="o", bufs=4))
        cpool = ctx.enter_context(tc.tile_pool(name="c", bufs=1))

        hA = hpool.tile([PT, KT, C], F16, tag="hA")
        hB = hpool.tile([PT, KT, C], F16, tag="hB")

        # All early DMAs round-robin across the shared SDMA engines at packet
        # granularity, so emission order ~= bandwidth share. The first matmul
        # needs stripe (w0, m=0) + x chunk 0; stripe m=1 is needed a few
        # hundred ns later; bias only at the first epilogue (~20us in).
        # Split the k=0 slices of stripes m=0,1 and the n=0 columns of x
        # chunk 0 into their own small DMAs: the first matmuls then gate on
        # ~120KB of receipts instead of ~800KB.
        wts0 = []
        for m in (0, 1):
            wt = wpool.tile([PT, D], F16, tag="wt", name=f"wt0_{m}")
            nc.scalar.dma_start(wt[:, 0:PT], w0[m, :, 0:PT])
            wts0.append(wt)
        nc.sync.dma_start(hA[:, 0, 0:TN], xT[0:PT, 0:TN])
        for m in (0, 1):
            nc.scalar.dma_start(wts0[m][:, PT:D], w0[m, :, PT:D])
        nc.sync.dma_start(hA[:, 0, TN:C], xT[0:PT, TN:C])
        for k in range(1, KT):
            nc.sync.dma_start(hA[:, k, :], xT[k * PT : (k + 1) * PT, :])
        bias_sb = cpool.tile([PT, 3 * MT], F32)
        nc.scalar.dma_start(bias_sb[:], bias[:])

        def relu_bias(out_ap, ps_ap, b_ap, on_dve):
            if on_dve:
                nc.vector.tensor_scalar(
                    out_ap, ps_ap, b_ap, 0.0,
                    mybir.AluOpType.add, mybir.AluOpType.max,
                )
            else:
                nc.scalar.activation(
                    out_ap, ps_ap,
                    mybir.ActivationFunctionType.Relu, bias=b_ap,
                )

        layers = [(w0, 0, hA, hB), (w1, 1, hB, hA), (w2, 2, hA, None)]
        for w_dram, li, h_in, h_out in layers:
            for mp in range(MT // 2):
                ms = (2 * mp, 2 * mp + 1)
                if li == 0 and mp == 0:
                    wts = wts0
                else:
                    wts = []
                    for m in ms:
                        wt = wpool.tile([PT, D], F16, tag="wt", name=f"wt{li}_{m}")
                        nc.scalar.dma_start(wt[:], w_dram[m])
                        wts.append(wt)
                pss = {
                    (m, n): pspool.tile([PT, TN], F32, tag="ps", name=f"ps{li}_{m}_{n}")
                    for m in ms
                    for n in range(NT)
                }

                def epilogue(mi, m, n):
                    b_ap = bias_sb[:, li * MT + m : li * MT + m + 1]
                    # alternate ACT/DVE so epilogues drain on two engines
                    on_dve = (n + mi) % 2 == 1
                    if h_out is not None:
                        relu_bias(
                            h_out[:, m, bass.ts(n, TN)], pss[(m, n)][:],
                            b_ap, on_dve,
                        )
                    else:
                        ot = opool.tile([PT, TN], F32, tag="ot", name=f"ot{m}_{n}")
                        relu_bias(ot[:], pss[(m, n)][:], b_ap, on_dve)
                        dma_eng = nc.sync if on_dve else nc.scalar
                        dma_eng.dma_start(
                            yT[m * PT : (m + 1) * PT, bass.ts(n, TN)], ot[:]
                        )

                if li == 0:
                    # k-outer: consume the streaming input chunks as they land
                    for k in range(KT):
                        for mi, m in enumerate(ms):
                            for n in range(NT):
                                nc.tensor.matmul(
                                    pss[(m, n)][:],
                                    wts[mi][:, k * PT : (k + 1) * PT],
                                    h_in[:, k, bass.ts(n, TN)],
                                    start=(k == 0),
                                    stop=(k == KT - 1),
                                    skip_group_check=True,
                                )
                    for mi, m in enumerate(ms):
                        for n in range(NT):
                            epilogue(mi, m, n)
                else:
                    # inputs resident: k-inner per tile, so each tile's
                    # epilogue (and final-layer out-DMA) fires as soon as its
                    # accumulation completes — the kernel tail drains one
                    # tile, not six
                    for mi, m in enumerate(ms):
                        for n in range(NT):
                            for k in range(KT):
                                nc.tensor.matmul(
                                    pss[(m, n)][:],
                                    wts[mi][:, k * PT : (k + 1) * PT],
                                    h_in[:, k, bass.ts(n, TN)],
                                    start=(k == 0),
                                    stop=(k == KT - 1),
                                )
                            if li == 2 and m == MT - 1 and n == NT - 1:
                                # very last tile: drain both halves in
                                # parallel on both engines and both DMA rings
                                b_ap = bias_sb[:, li * MT + m : li * MT + m + 1]
                                h2 = TN // 2
                                ot = opool.tile([PT, TN], F32, tag="ot", name="ot_last")
                                ps_ap = pss[(m, n)]
                                rows = slice(m * PT, (m + 1) * PT)
                                nc.scalar.activation(
                                    ot[:, 0:h2], ps_ap[:, 0:h2],
                                    mybir.ActivationFunctionType.Relu, bias=b_ap,
                                )
                                nc.vector.tensor_scalar(
                                    ot[:, h2:TN], ps_ap[:, h2:TN], b_ap, 0.0,
                                    mybir.AluOpType.add, mybir.AluOpType.max,
                                )
                                nc.sync.dma_start(
                                    yT[rows, n * TN : n * TN + h2], ot[:, 0:h2]
                                )
                                nc.scalar.dma_start(
                                    yT[rows, n * TN + h2 : (n + 1) * TN], ot[:, h2:TN]
                                )
                            else:
                                epilogue(mi, m, n)
    nc.compile()
    return nc


def _apportion_cores(counts):
    """Assign 8 cores to 4 leaves ~proportionally to token counts.
    Returns list of core counts per leaf (sums to N_CORES; 0 only for empty
    leaves). Greedy: repeatedly hand a core to the leaf with max load/core."""
    alive = [l for l in range(4) if counts[l] > 0]
    n = {l: 1 for l in alive}
    for _ in range(N_CORES - len(alive)):
        l = max(alive, key=lambda l: counts[l] / n[l])
        n[l] += 1
    return [n.get(l, 0) for l in range(4)]


def kernel(x, W0, b0, W1, b1, W2, b2, path_mask):
    global last_results
    x = np.asarray(x, dtype=np.float32)
    path_mask = np.asarray(path_mask)
    W0, b0, W1, b1, W2, b2 = (
        np.asarray(a, dtype=np.float32) for a in (W0, b0, W1, b1, W2, b2)
    )
    B = x.shape[0]

    bit0 = path_mask[:, 0].astype(np.int64)
    bit1 = path_mask[:, 1].astype(np.int64)
    leaf = 2 * bit0 + bit1
    order = np.argsort(leaf, kind="stable")
    counts = np.bincount(leaf, minlength=4)

    per_leaf = _apportion_cores(counts)
    # contiguous chunks of the leaf-sorted order per core
    groups = []      # list of (leaf, index-array) per core
    start = 0
    for l in range(4):
        cnt = int(counts[l])
        tok = order[start : start + cnt]
        start += cnt
        nl = per_leaf[l]
        if nl == 0:
            continue
        bounds = [round(i * cnt / nl) for i in range(nl + 1)]
        for i in range(nl):
            groups.append((l, tok[bounds[i] : bounds[i + 1]]))
    while len(groups) < N_CORES:  # only if some leaf was empty and slots remain
        groups.append((0, np.zeros(0, dtype=np.int64)))

    maxg = max(len(g[1]) for g in groups)
    TN, NT, C = _tiling(maxg)

    if C not in _compiled:
        _compiled[C] = _build(C, TN, NT)
    nc = _compiled[C]

    w_prepped = {}  # cache per (matrix id)
    def wp(tag, W):
        if tag not in w_prepped:
            w_prepped[tag] = _prep_weight(W)
        return w_prepped[tag]

    xb = x.astype(NP_F16)
    in_maps = []
    for l, tok in groups:
        xTg = np.zeros((D, C), dtype=NP_F16)
        if len(tok):
            xTg[:, : len(tok)] = xb[tok].T
        in_maps.append(
            {
                "xT": xTg,
                "w0": wp("w0", W0),
                "w1": wp(("w1", l // 2), W1[l // 2]),
                "w2": wp(("w2", l), W2[l]),
                "bias": _prep_bias(b0, b1[l // 2], b2[l]),
            }
        )

    last_results = run_bass_kernel_spmd(nc, in_maps, core_ids=list(range(N_CORES)))

    y = np.empty((B, D), dtype=np.float32)
    for (l, tok), res in zip(groups, last_results.results):
        if len(tok):
            y[tok] = res["yT"][:, : len(tok)].T
    return y
